# revision 1
# baseline (speedup 1.0000x reference)
"""Trainium2 Bass kernel for 16-head MHA (B=2, L=2048, D=1024), 8 NeuronCores.

Sharding: 8 cores = 4 head-groups x 2 batches. Core c handles head group
hg = c // 2 (4 heads = 256 of the 1024 projection columns) for batch
b = c % 2. Per core, for its batch:
  - qhT/khT/vhT slices (256, 2048) [head-dim on partitions, seq free],
    fp16 operands, fp32 PSUM accumulation.
  - vhT is DMA-transposed into an augmented V layout: per key tile, 4
    head blocks of [64 dims | ones column], so the P@V matmul (M=65
    stationary) also produces the softmax row sums.
  - attention in 8 rounds of (head-pair, query-quarter): S_T scores
    (keys on partitions, 2-head row-tiling), one wide exp on ScalarE
    (softmax scale folded into the activation's affine), P@V
    accumulation over key tiles, then reciprocal + K=1-matmul broadcast
    + multiply normalization off the critical path (double-buffered
    PSUM).
  - row-packed output projection against Wo -> partial (2048, 1024).
Host sums the 4 head-group partials per batch and adds bo.
"""

import dataclasses
import sys

sys.path.insert(0, "/opt/trn_rl_repo")

import numpy as np

import concourse.bass as bass  # noqa: F401  (registers types)
import concourse.mybir as mybir
import concourse.tile as tile
from concourse import bacc
from concourse import library_config
from concourse.bass import ds, ts
from concourse.bass_utils import run_bass_kernel_spmd

F32 = mybir.dt.float32
F16 = mybir.dt.float16
I16 = mybir.dt.int16
AF = mybir.ActivationFunctionType

# Schraudolph exp on DVE: fp16 bitpattern y = round(A*raw_score + B) so that
# int16->fp16 bitcast approximates exp(0.125*s) within +-3%. Applied to a
# minority of key tiles to offload the ACT engine (softmax renormalizes the
# shared bias; residual error ~6e-3 at 4/16 tiles, tolerance 2e-2).
EXP_A = 1024.0 / 0.6931471805599453 * 0.125
EXP_B = 15360.0 - 58.0
DVE_EXP_TILES = frozenset((5, 9, 13))

D = 1024          # model dim
L = 2048          # sequence length
B = 2             # batch
NH = 16           # total heads
HD = 64           # head dim
HS = 256          # head-slice columns per core (4 heads)
HC = HD + 1       # head block width in the augmented V layout
KT = D // 128     # 8 contraction tiles for projections
LT = L // 128     # 16 key tiles
N_CORES = 8

_PROGRAM = None


def _build_program():
    nc = bacc.Bacc(
        "TRN2",
        target_bir_lowering=False,
        debug=False,
        enable_asserts=False,
        num_devices=N_CORES,
    )
    xqT = nc.dram_tensor("xqT", (D, L), F16, kind="ExternalInput").ap()
    xkT = nc.dram_tensor("xkT", (D, L), F16, kind="ExternalInput").ap()
    xvT = nc.dram_tensor("xvT", (D, L), F16, kind="ExternalInput").ap()
    wqT = nc.dram_tensor("wqT", (D, HS), F16, kind="ExternalInput").ap()
    wkT = nc.dram_tensor("wkT", (D, HS), F16, kind="ExternalInput").ap()
    wvT = nc.dram_tensor("wvT", (D, HS), F16, kind="ExternalInput").ap()
    woT = nc.dram_tensor("woT", (HS, D), F16, kind="ExternalInput").ap()
    bqkv = nc.dram_tensor("bqkv", (128, 6), F32, kind="ExternalInput").ap()
    onesv = nc.dram_tensor("onesv", (128, LT, 4), F16, kind="ExternalInput").ap()
    onesr = nc.dram_tensor("onesr", (1, 64), F16, kind="ExternalInput").ap()
    ident = nc.dram_tensor("ident", (128, 128), F16, kind="ExternalInput").ap()
    out = nc.dram_tensor("out", (L, D), F16, kind="ExternalOutput").ap()

    with tile.TileContext(nc) as tc:
        _emit(nc, tc, xqT, xkT, xvT, wqT, wkT, wvT, woT, bqkv, onesv, onesr, ident, out)
    nc.compile()
    return nc


def _emit(nc, tc, xqT, xkT, xvT, wqT, wkT, wvT, woT, bqkv, onesv, onesr, ident, out):
    with (
        tc.tile_pool(name="const", bufs=1) as constp,
        tc.tile_pool(name="wpool", bufs=1) as wpool,
        tc.tile_pool(name="proj", bufs=1) as projp,
        tc.tile_pool(name="xt", bufs=6) as xtp,
        tc.tile_pool(name="pt", bufs=4) as ptp,
        tc.tile_pool(name="small", bufs=4) as smallp,
        tc.tile_pool(name="outsb", bufs=6) as outp,
    ):
        # --- constants ---
        bqkv_sb = constp.tile([128, 6], F32)
        nc.sync.dma_start(bqkv_sb[:], bqkv)
        onesr_sb = constp.tile([1, 64], F16)
        nc.sync.dma_start(onesr_sb[:], onesr)
        ident_sb = constp.tile([128, 128], F16)
        nc.sync.dma_start(ident_sb[:], ident)

        # --- persistent activations ---
        qh_sb = [projp.tile([128, L], F16, tag=f"qh{m}", name=f"qh{m}") for m in range(2)]
        kh_sb = [projp.tile([128, L], F16, tag=f"kh{m}", name=f"kh{m}") for m in range(2)]
        vt_sb = [projp.tile([128, L], F16, tag=f"vt{m}", name=f"vt{m}") for m in range(2)]
        # augmented V: per key tile, 4 head blocks of [64 dims | ones col]
        vh_sb = projp.tile([128, LT, 4 * HC], F16, tag="vh", name="vh")
        on_sb = [
            [projp.tile([128, 512], F16, tag=f"on{p}q{q_}", name=f"on{p}q{q_}")
             for q_ in range(4)]
            for p in range(2)
        ]
        vh4 = vh_sb[:].rearrange("p t (h c) -> p t h c", c=HC)

        def load_w(name, src):
            t = wpool.tile([128, KT, HS], F16, tag=name, name=name)
            nc.sync.dma_start(t[:], src.rearrange("(t p) c -> p t c", p=128))
            return t

        # --- phase 1: q/k/v projections (head-dim on partitions) ---
        # weight loads are emitted just-in-time so the xq stream + q-proj
        # start as early as possible
        with tc.tile_pool(name="pjps", bufs=2, space="PSUM") as pA:
            for which, (xdram, wname, wsrc, dst, bias0) in enumerate((
                (xvT, "wv", wvT, vt_sb, 4),
                (xqT, "wq", wqT, qh_sb, 0),
                (xkT, "wk", wkT, kh_sb, 2),
            )):
                w_sb = load_w(wname, wsrc)
                ps = [pA.tile([128, L], F32, tag="pj", name="pjps") for _ in range(2)]
                for t in range(KT):
                    xt_ = xtp.tile([128, L], F16, tag="xt")
                    nc.sync.dma_start(xt_[:, ds(0, 1024)], xdram[ts(t, 128), ds(0, 1024)])
                    nc.sync.dma_start(xt_[:, ds(1024, 1024)], xdram[ts(t, 128), ds(1024, 1024)])
                    for m in range(2):
                        for n in range(4):
                            nc.tensor.matmul(
                                ps[m][:, ts(n, 512)],
                                lhsT=w_sb[:, t, ts(m, 128)],
                                rhs=xt_[:, ts(n, 512)],
                                start=(t == 0),
                                stop=(t == KT - 1),
                            )
                for m in range(2):
                    # bias add on ACT (idle during proj) keeps DVE free
                    nc.scalar.activation(
                        dst[m][:], ps[m][:], AF.Identity,
                        bias=bqkv_sb[:, ds(bias0 + m, 1)],
                    )
                if which == 0:
                    # V transposed through the PE (transpose-mode matmul into
                    # recycled projection PSUM, fp16-bitcast slices), with ACT
                    # draining PSUM->vh. Runs behind the k/q projections; no
                    # DMA-transpose traffic at all.
                    nc.sync.dma_start(vh4[:, :, :, ds(HD, 1)], onesv.unsqueeze(-1))
                    trps = pA.tile([128, L], F32, tag="pj", name="trps")
                    for p in range(2):
                        for m in range(LT):
                            slot = trps[:, ts(p * LT + m, 64)].bitcast(F16)
                            nc.tensor.transpose(
                                slot, vt_sb[p][:, ts(m, 128)], ident_sb[:]
                            )
                            nc.scalar.copy(
                                vh4[:, m, ds(2 * p, 2), ds(0, HD)],
                                slot.rearrange("p (h c) -> p h c", c=HD),
                            )

        # late weights (not needed until out-proj)
        wo_sb = []
        for p in range(2):
            t = wpool.tile([128, D], F16, tag=f"wo{p}", name=f"wo{p}")
            nc.sync.dma_start(t[:], woT[ts(p, 128), :])
            wo_sb.append(t)

        # --- phase 2: attention, 8 rounds of (query-quarter, head-pair) ---
        with tc.tile_pool(name="atps", bufs=2, space="PSUM") as pC:

            def norm_recip(o_ps):
                # row 64 of each O bank holds the exp row sums. reciprocal
                # into partition 0 of rb, then a log2 DMA-doubling broadcast
                # across partitions (SBUF-only, on the gpsimd queue: leaves
                # the hot PSUM slots and the SP queue alone). The multiplies
                # are emitted several tiles later (norm_mult) so the DVE
                # never blocks in-order on this DMA chain.
                rbs = []
                for h2 in range(2):
                    ot = o_ps[h2]
                    rb = smallp.tile([64, 512], F16, tag=f"rb{h2}", name="rb")
                    rb0 = smallp.tile([1, 512], F32, tag=f"rb0{h2}", name="rb0")
                    rbA = smallp.tile([1, 512], F32, tag=f"rbA{h2}", name="rbA")
                    # custom-DVE approx recip is invisible to the tile dep
                    # tracker: sandwich it between tracked DVE ops (same
                    # engine => program order) for input/output safety
                    nc.vector.tensor_copy(rb0[:], ot[ds(HD, 1), :])
                    nc.vector.reciprocal_approx_fast(rbA[:], rb0[:])
                    nc.vector.tensor_copy(rb[ds(0, 1), :], rbA[:])
                    # 3-hop 4x broadcast tree (free-dim stride-0 repeat):
                    # ~3 DMA latencies instead of 6 for the old 2x doubling
                    base = 1
                    while base < 64:
                        srcap = rb[ds(0, base), :].unsqueeze(1)
                        srcap = dataclasses.replace(
                            srcap, ap=[srcap.ap[0], [0, 3], srcap.ap[2]]
                        )
                        nc.sync.dma_start(rb[ds(base, 3 * base), :], srcap)
                        base *= 4
                    rbs.append(rb)
                return rbs

            def norm_mult(p, qq, o_ps, rbs, h2):
                ot = o_ps[h2]
                if h2 == 0:
                    nc.vector.tensor_mul(
                        on_sb[p][qq][ds(0, HD), :],
                        ot[ds(0, HD), :],
                        rbs[0][:],
                    )
                else:
                    om = smallp.tile([64, 512], F16, tag="om", name="om")
                    nc.vector.tensor_mul(om[:], ot[ds(0, HD), :], rbs[1][:])
                    # partition shift 0-63 -> 64-127 via DMA
                    nc.sync.dma_start(on_sb[p][qq][ds(64, HD), :], om[:])

            def emit_scores_exp(p, qq, t):
                s_ps = pC.tile([128, 1024], F32, tag="s", name="s_ps")
                for h2 in range(2):
                    nc.tensor.matmul(
                        s_ps[:, ts(h2, 512)],
                        lhsT=kh_sb[p][ds(h2 * 64, 64), ts(t, 128)],
                        rhs=qh_sb[p][ds(h2 * 64, 64), ts(qq, 512)],
                        start=True,
                        stop=True,
                        tile_position=(h2 * 64, 0),
                    )
                p_t = ptp.tile([128, 1024], F16, tag="pt", name="p_t")
                if t in DVE_EXP_TILES:
                    # Schraudolph bit-trick exp on DVE: offloads the ACT
                    # engine, which otherwise paces the whole attention loop
                    nc.vector.tensor_scalar(
                        p_t[:].bitcast(I16), s_ps[:], EXP_A, EXP_B,
                        mybir.AluOpType.mult, mybir.AluOpType.add,
                    )
                else:
                    nc.scalar.activation(p_t[:], s_ps[:], AF.Exp, scale=0.125)
                return p_t

            def emit_pv(p, o_ps, p_t, t):
                for h2 in range(2):
                    nc.tensor.matmul(
                        o_ps[h2][ds(0, HC), :],
                        lhsT=vh_sb[:, t, ds((2 * p + h2) * HC, HC)],
                        rhs=p_t[:, ts(h2, 512)],
                        start=(t == 0),
                        stop=(t == LT - 1),
                    )

            # software-pipelined emission: P@V for key tile t is emitted
            # after scores/exp for t+2 (2-tile lag), so the dependency loop
            # exp(t) -> pv(t) -> scores(t+2) -> exp(t+2) never throttles the
            # PE: scores for t+2 only wait on exp(t) via PSUM buffer reuse,
            # and that wait is hidden behind two tiles of PE work.
            # Normalization of the previous round and the output projection
            # of the previous quarter are emitted mid-round so their latency
            # hides behind the streaming loop.
            pending_norm = None
            for qq in range(4):
                for p in range(2):
                    o_ps = [
                        pC.tile([128, 512], F32, tag=f"o{h2}", name=f"o{h2}")
                        for h2 in range(2)
                    ]
                    pipe = [emit_scores_exp(p, qq, 0), emit_scores_exp(p, qq, 1)]
                    rbs = None
                    for t in range(2, LT):
                        pipe.append(emit_scores_exp(p, qq, t))
                        emit_pv(p, o_ps, pipe.pop(0), t - 2)
                        if pending_norm is not None:
                            if t == 3:
                                rbs = norm_recip(pending_norm[2])
                            elif t == 12:
                                norm_mult(*pending_norm, rbs, 0)
                            elif t == 14:
                                norm_mult(*pending_norm, rbs, 1)
                                pending_norm = None
                    emit_pv(p, o_ps, pipe.pop(0), LT - 2)
                    emit_pv(p, o_ps, pipe.pop(0), LT - 1)
                    pending_norm = (p, qq, o_ps)
            # last round: drain O out of PSUM immediately (the multiplies
            # would otherwise pin the o-banks until the reciprocal chain
            # lands, gating the out-projection's PSUM reuse)
            rbs = norm_recip(pending_norm[2])
            oc_sb = []
            for h2 in range(2):
                t_ = smallp.tile([64, 512], F32, tag=f"ocs{h2}", name="ocs")
                nc.vector.tensor_copy(t_[:], pending_norm[2][h2][ds(0, HD), :])
                oc_sb.append(t_)
            p_, qq_ = pending_norm[0], pending_norm[1]
            nc.vector.tensor_mul(
                on_sb[p_][qq_][ds(0, HD), :], oc_sb[0][:], rbs[0][:]
            )
            om_ = smallp.tile([64, 512], F16, tag="om", name="om")
            nc.vector.tensor_mul(om_[:], oc_sb[1][:], rbs[1][:])
            nc.sync.dma_start(on_sb[p_][qq_][ds(64, HD), :], om_[:])

        # --- phase 3: output projection tail ---
        # Each head-pair's two heads are stacked on partitions 0-127 of
        # on_sb[p] / wo_sb[p], so one K=128 matmul per pair contracts over
        # both heads at once. PSUM->SBUF copies ride mostly on ACT (the DVE
        # is finishing the last round's norm), the store DMAs are fp16.
        with tc.tile_pool(name="opps", bufs=4, space="PSUM") as pD:
            for qt in range(LT):
                out_t = outp.tile([128, D], F16, tag="ot", name="out_t")
                psA = pD.tile([128, 1024], F32, tag="opA", name="psA")
                for oc in range(2):
                    for p in range(2):
                        nc.tensor.matmul(
                            psA[:, ts(oc, 512)],
                            lhsT=on_sb[p][qt // 4][:, ts(qt % 4, 128)],
                            rhs=wo_sb[p][:, ts(oc, 512)],
                            start=(p == 0),
                            stop=(p == 1),
                        )
                    if oc == 0 and qt >= 6:
                        nc.vector.tensor_copy(out_t[:, ts(oc, 512)], psA[:, ts(oc, 512)])
                    else:
                        nc.scalar.copy(out_t[:, ts(oc, 512)], psA[:, ts(oc, 512)])
                nc.sync.dma_start(out[ts(qt, 128), :], out_t[:])


def get_program():
    global _PROGRAM
    if _PROGRAM is None:
        _PROGRAM = _build_program()
    return _PROGRAM


def prepare_in_maps(q, k, v, Wq, bq, Wk, bk, Wv, bv, Wo, bo):
    """Build the 8 per-core input dicts (host-side slicing/transposes)."""
    q = np.asarray(q, dtype=np.float32)
    k = np.asarray(k, dtype=np.float32)
    v = np.asarray(v, dtype=np.float32)
    xT = {}
    for b in range(B):
        xT[("q", b)] = np.ascontiguousarray(q[b].T).astype(np.float16)
        xT[("k", b)] = np.ascontiguousarray(k[b].T).astype(np.float16)
        xT[("v", b)] = np.ascontiguousarray(v[b].T).astype(np.float16)
    ones_v = np.ones((128, LT, 4), dtype=np.float16)
    ones_r = np.ones((1, 64), dtype=np.float16)
    ident_m = np.eye(128, dtype=np.float16)
    in_maps = []
    for c in range(N_CORES):
        hg, b = c // 2, c % 2
        hs = hg * HS
        bq_s = np.asarray(bq, np.float32)[hs : hs + HS]
        bk_s = np.asarray(bk, np.float32)[hs : hs + HS]
        bv_s = np.asarray(bv, np.float32)[hs : hs + HS]
        bqkv_m = np.stack(
            [
                bq_s[0:128],
                bq_s[128:256],
                bk_s[0:128],
                bk_s[128:256],
                bv_s[0:128],
                bv_s[128:256],
            ],
            axis=1,
        )
        in_maps.append(
            {
                "xqT": xT[("q", b)],
                "xkT": xT[("k", b)],
                "xvT": xT[("v", b)],
                "wqT": np.asarray(Wq, np.float32)[hs : hs + HS, :].T.astype(np.float16),
                "wkT": np.asarray(Wk, np.float32)[hs : hs + HS, :].T.astype(np.float16),
                "wvT": np.asarray(Wv, np.float32)[hs : hs + HS, :].T.astype(np.float16),
                "woT": np.asarray(Wo, np.float32)[:, hs : hs + HS].T.astype(np.float16),
                "bqkv": np.ascontiguousarray(bqkv_m),
                "onesv": ones_v,
                "onesr": ones_r,
                "ident": ident_m,
            }
        )
    return in_maps


def combine_outputs(results, bo):
    """Sum head-group partials per batch and add the output bias."""
    bo = np.asarray(bo, np.float32)
    full = np.zeros((B, L, D), dtype=np.float32)
    for c in range(N_CORES):
        hg, b = c // 2, c % 2
        full[b] += np.asarray(results[c]["out"], dtype=np.float32)
    full += bo
    return full


def run(inputs, trace=False, trace_cores=None):
    nc = get_program()
    in_maps = prepare_in_maps(**inputs)
    res = run_bass_kernel_spmd(
        nc,
        in_maps,
        core_ids=list(range(N_CORES)),
        trace=trace,
        trace_cores=trace_cores,
    )
    out = combine_outputs(res.results, inputs["bo"])
    return out, res


def kernel(**inputs):
    out, _ = run(inputs, trace=False)
    return out



# revision 2
# speedup vs baseline: 1.0036x; 1.0036x over previous
"""Trainium2 Bass kernel for 16-head MHA (B=2, L=2048, D=1024), 8 NeuronCores.

Sharding: 8 cores = 4 head-groups x 2 batches. Core c handles head group
hg = c // 2 (4 heads = 256 of the 1024 projection columns) for batch
b = c % 2. Per core, for its batch:
  - qhT/khT/vhT slices (256, 2048) [head-dim on partitions, seq free],
    fp16 operands, fp32 PSUM accumulation.
  - vhT is DMA-transposed into an augmented V layout: per key tile, 4
    head blocks of [64 dims | ones column], so the P@V matmul (M=65
    stationary) also produces the softmax row sums.
  - attention in 8 rounds of (head-pair, query-quarter): S_T scores
    (keys on partitions, 2-head row-tiling), one wide exp on ScalarE
    (softmax scale folded into the activation's affine), P@V
    accumulation over key tiles, then reciprocal + K=1-matmul broadcast
    + multiply normalization off the critical path (double-buffered
    PSUM).
  - row-packed output projection against Wo -> partial (2048, 1024).
Host sums the 4 head-group partials per batch and adds bo.
"""

import dataclasses
import sys

sys.path.insert(0, "/opt/trn_rl_repo")

import numpy as np

import concourse.bass as bass  # noqa: F401  (registers types)
import concourse.mybir as mybir
import concourse.tile as tile
from concourse import bacc
from concourse import library_config
from concourse.bass import ds, ts
from concourse.bass_utils import run_bass_kernel_spmd

F32 = mybir.dt.float32
F16 = mybir.dt.float16
I16 = mybir.dt.int16
AF = mybir.ActivationFunctionType

# Schraudolph exp on DVE: fp16 bitpattern y = round(A*raw_score + B) so that
# int16->fp16 bitcast approximates exp(0.125*s) within +-3%. Applied to a
# minority of key tiles to offload the ACT engine (softmax renormalizes the
# shared bias; residual error ~6e-3 at 4/16 tiles, tolerance 2e-2).
EXP_A = 1024.0 / 0.6931471805599453 * 0.125
EXP_B = 15360.0 - 58.0
DVE_EXP_TILES = frozenset((2, 5, 8, 11, 13, 15))

D = 1024          # model dim
L = 2048          # sequence length
B = 2             # batch
NH = 16           # total heads
HD = 64           # head dim
HS = 256          # head-slice columns per core (4 heads)
HC = HD + 1       # head block width in the augmented V layout
KT = D // 128     # 8 contraction tiles for projections
LT = L // 128     # 16 key tiles
N_CORES = 8

_PROGRAM = None


def _build_program():
    nc = bacc.Bacc(
        "TRN2",
        target_bir_lowering=False,
        debug=False,
        enable_asserts=False,
        num_devices=N_CORES,
    )
    xqT = nc.dram_tensor("xqT", (D, L), F16, kind="ExternalInput").ap()
    xkT = nc.dram_tensor("xkT", (D, L), F16, kind="ExternalInput").ap()
    xvT = nc.dram_tensor("xvT", (D, L), F16, kind="ExternalInput").ap()
    wqT = nc.dram_tensor("wqT", (D, HS), F16, kind="ExternalInput").ap()
    wkT = nc.dram_tensor("wkT", (D, HS), F16, kind="ExternalInput").ap()
    wvT = nc.dram_tensor("wvT", (D, HS), F16, kind="ExternalInput").ap()
    woT = nc.dram_tensor("woT", (HS, D), F16, kind="ExternalInput").ap()
    bqkv = nc.dram_tensor("bqkv", (128, 6), F32, kind="ExternalInput").ap()
    onesv = nc.dram_tensor("onesv", (128, LT, 4), F16, kind="ExternalInput").ap()
    onesr = nc.dram_tensor("onesr", (1, 64), F16, kind="ExternalInput").ap()
    ident = nc.dram_tensor("ident", (128, 128), F16, kind="ExternalInput").ap()
    out = nc.dram_tensor("out", (L, D), F16, kind="ExternalOutput").ap()

    with tile.TileContext(nc) as tc:
        _emit(nc, tc, xqT, xkT, xvT, wqT, wkT, wvT, woT, bqkv, onesv, onesr, ident, out)
    nc.compile()
    return nc


def _emit(nc, tc, xqT, xkT, xvT, wqT, wkT, wvT, woT, bqkv, onesv, onesr, ident, out):
    with (
        tc.tile_pool(name="const", bufs=1) as constp,
        tc.tile_pool(name="wpool", bufs=1) as wpool,
        tc.tile_pool(name="proj", bufs=1) as projp,
        tc.tile_pool(name="xt", bufs=6) as xtp,
        tc.tile_pool(name="pt", bufs=4) as ptp,
        tc.tile_pool(name="small", bufs=4) as smallp,
        tc.tile_pool(name="outsb", bufs=6) as outp,
    ):
        # --- constants ---
        bqkv_sb = constp.tile([128, 6], F32)
        nc.sync.dma_start(bqkv_sb[:], bqkv)
        onesr_sb = constp.tile([1, 64], F16)
        nc.sync.dma_start(onesr_sb[:], onesr)
        ident_sb = constp.tile([128, 128], F16)
        nc.sync.dma_start(ident_sb[:], ident)

        # --- persistent activations ---
        qh_sb = [projp.tile([128, L], F16, tag=f"qh{m}", name=f"qh{m}") for m in range(2)]
        kh_sb = [projp.tile([128, L], F16, tag=f"kh{m}", name=f"kh{m}") for m in range(2)]
        vt_sb = [projp.tile([128, L], F16, tag=f"vt{m}", name=f"vt{m}") for m in range(2)]
        # augmented V: per key tile, 4 head blocks of [64 dims | ones col]
        vh_sb = projp.tile([128, LT, 4 * HC], F16, tag="vh", name="vh")
        on_sb = [
            [projp.tile([128, 512], F16, tag=f"on{p}q{q_}", name=f"on{p}q{q_}")
             for q_ in range(4)]
            for p in range(2)
        ]
        vh4 = vh_sb[:].rearrange("p t (h c) -> p t h c", c=HC)

        def load_w(name, src):
            t = wpool.tile([128, KT, HS], F16, tag=name, name=name)
            nc.sync.dma_start(t[:], src.rearrange("(t p) c -> p t c", p=128))
            return t

        # --- phase 1: q/k/v projections (head-dim on partitions) ---
        # weight loads are emitted just-in-time so the xq stream + q-proj
        # start as early as possible
        with tc.tile_pool(name="pjps", bufs=2, space="PSUM") as pA:
            for which, (xdram, wname, wsrc, dst, bias0) in enumerate((
                (xvT, "wv", wvT, vt_sb, 4),
                (xqT, "wq", wqT, qh_sb, 0),
                (xkT, "wk", wkT, kh_sb, 2),
            )):
                w_sb = load_w(wname, wsrc)
                ps = [pA.tile([128, L], F32, tag="pj", name="pjps") for _ in range(2)]
                for t in range(KT):
                    xt_ = xtp.tile([128, L], F16, tag="xt")
                    nc.sync.dma_start(xt_[:, ds(0, 1024)], xdram[ts(t, 128), ds(0, 1024)])
                    nc.sync.dma_start(xt_[:, ds(1024, 1024)], xdram[ts(t, 128), ds(1024, 1024)])
                    for m in range(2):
                        for n in range(4):
                            nc.tensor.matmul(
                                ps[m][:, ts(n, 512)],
                                lhsT=w_sb[:, t, ts(m, 128)],
                                rhs=xt_[:, ts(n, 512)],
                                start=(t == 0),
                                stop=(t == KT - 1),
                            )
                for m in range(2):
                    # bias add on ACT (idle during proj) keeps DVE free
                    nc.scalar.activation(
                        dst[m][:], ps[m][:], AF.Identity,
                        bias=bqkv_sb[:, ds(bias0 + m, 1)],
                    )
                if which == 0:
                    # V transposed through the PE (transpose-mode matmul into
                    # recycled projection PSUM, fp16-bitcast slices), with ACT
                    # draining PSUM->vh. Runs behind the k/q projections; no
                    # DMA-transpose traffic at all.
                    nc.sync.dma_start(vh4[:, :, :, ds(HD, 1)], onesv.unsqueeze(-1))
                    trps = pA.tile([128, L], F32, tag="pj", name="trps")
                    for p in range(2):
                        for m in range(LT):
                            slot = trps[:, ts(p * LT + m, 64)].bitcast(F16)
                            nc.tensor.transpose(
                                slot, vt_sb[p][:, ts(m, 128)], ident_sb[:]
                            )
                            nc.scalar.copy(
                                vh4[:, m, ds(2 * p, 2), ds(0, HD)],
                                slot.rearrange("p (h c) -> p h c", c=HD),
                            )

        # late weights (not needed until out-proj)
        wo_sb = []
        for p in range(2):
            t = wpool.tile([128, D], F16, tag=f"wo{p}", name=f"wo{p}")
            nc.sync.dma_start(t[:], woT[ts(p, 128), :])
            wo_sb.append(t)

        # --- phase 2: attention, 8 rounds of (query-quarter, head-pair) ---
        with tc.tile_pool(name="atps", bufs=2, space="PSUM") as pC:

            def norm_recip(o_ps):
                # row 64 of each O bank holds the exp row sums. reciprocal
                # into partition 0 of rb, then a log2 DMA-doubling broadcast
                # across partitions (SBUF-only, on the gpsimd queue: leaves
                # the hot PSUM slots and the SP queue alone). The multiplies
                # are emitted several tiles later (norm_mult) so the DVE
                # never blocks in-order on this DMA chain.
                rbs = []
                for h2 in range(2):
                    ot = o_ps[h2]
                    rb = smallp.tile([64, 512], F16, tag=f"rb{h2}", name="rb")
                    rb0 = smallp.tile([1, 512], F32, tag=f"rb0{h2}", name="rb0")
                    rbA = smallp.tile([1, 512], F32, tag=f"rbA{h2}", name="rbA")
                    # custom-DVE approx recip is invisible to the tile dep
                    # tracker: sandwich it between tracked DVE ops (same
                    # engine => program order) for input/output safety
                    nc.vector.tensor_copy(rb0[:], ot[ds(HD, 1), :])
                    nc.vector.reciprocal_approx_fast(rbA[:], rb0[:])
                    nc.vector.tensor_copy(rb[ds(0, 1), :], rbA[:])
                    # 3-hop 4x broadcast tree (free-dim stride-0 repeat):
                    # ~3 DMA latencies instead of 6 for the old 2x doubling
                    base = 1
                    while base < 64:
                        srcap = rb[ds(0, base), :].unsqueeze(1)
                        srcap = dataclasses.replace(
                            srcap, ap=[srcap.ap[0], [0, 3], srcap.ap[2]]
                        )
                        nc.sync.dma_start(rb[ds(base, 3 * base), :], srcap)
                        base *= 4
                    rbs.append(rb)
                return rbs

            def norm_mult(p, qq, o_ps, rbs, h2):
                ot = o_ps[h2]
                if h2 == 0:
                    nc.vector.tensor_mul(
                        on_sb[p][qq][ds(0, HD), :],
                        ot[ds(0, HD), :],
                        rbs[0][:],
                    )
                else:
                    om = smallp.tile([64, 512], F16, tag="om", name="om")
                    nc.vector.tensor_mul(om[:], ot[ds(0, HD), :], rbs[1][:])
                    # partition shift 0-63 -> 64-127 via DMA
                    nc.sync.dma_start(on_sb[p][qq][ds(64, HD), :], om[:])

            def emit_scores_exp(p, qq, t):
                s_ps = pC.tile([128, 1024], F32, tag="s", name="s_ps")
                for h2 in range(2):
                    nc.tensor.matmul(
                        s_ps[:, ts(h2, 512)],
                        lhsT=kh_sb[p][ds(h2 * 64, 64), ts(t, 128)],
                        rhs=qh_sb[p][ds(h2 * 64, 64), ts(qq, 512)],
                        start=True,
                        stop=True,
                        tile_position=(h2 * 64, 0),
                    )
                p_t = ptp.tile([128, 1024], F16, tag="pt", name="p_t")
                if t in DVE_EXP_TILES:
                    # Schraudolph bit-trick exp on DVE: offloads the ACT
                    # engine, which otherwise paces the whole attention loop
                    nc.vector.tensor_scalar(
                        p_t[:].bitcast(I16), s_ps[:], EXP_A, EXP_B,
                        mybir.AluOpType.mult, mybir.AluOpType.add,
                    )
                else:
                    nc.scalar.activation(p_t[:], s_ps[:], AF.Exp, scale=0.125)
                return p_t

            def emit_pv(p, o_ps, p_t, t):
                for h2 in range(2):
                    nc.tensor.matmul(
                        o_ps[h2][ds(0, HC), :],
                        lhsT=vh_sb[:, t, ds((2 * p + h2) * HC, HC)],
                        rhs=p_t[:, ts(h2, 512)],
                        start=(t == 0),
                        stop=(t == LT - 1),
                    )

            # software-pipelined emission: P@V for key tile t is emitted
            # after scores/exp for t+2 (2-tile lag), so the dependency loop
            # exp(t) -> pv(t) -> scores(t+2) -> exp(t+2) never throttles the
            # PE: scores for t+2 only wait on exp(t) via PSUM buffer reuse,
            # and that wait is hidden behind two tiles of PE work.
            # Normalization of the previous round and the output projection
            # of the previous quarter are emitted mid-round so their latency
            # hides behind the streaming loop.
            pending_norm = None
            for qq in range(4):
                for p in range(2):
                    o_ps = [
                        pC.tile([128, 512], F32, tag=f"o{h2}", name=f"o{h2}")
                        for h2 in range(2)
                    ]
                    pipe = [emit_scores_exp(p, qq, 0), emit_scores_exp(p, qq, 1)]
                    rbs = None
                    for t in range(2, LT):
                        pipe.append(emit_scores_exp(p, qq, t))
                        emit_pv(p, o_ps, pipe.pop(0), t - 2)
                        if pending_norm is not None:
                            if t == 3:
                                rbs = norm_recip(pending_norm[2])
                            elif t == 12:
                                norm_mult(*pending_norm, rbs, 0)
                            elif t == 14:
                                norm_mult(*pending_norm, rbs, 1)
                                pending_norm = None
                    emit_pv(p, o_ps, pipe.pop(0), LT - 2)
                    emit_pv(p, o_ps, pipe.pop(0), LT - 1)
                    pending_norm = (p, qq, o_ps)
            # last round: drain O out of PSUM immediately (the multiplies
            # would otherwise pin the o-banks until the reciprocal chain
            # lands, gating the out-projection's PSUM reuse)
            rbs = norm_recip(pending_norm[2])
            oc_sb = []
            for h2 in range(2):
                t_ = smallp.tile([64, 512], F32, tag=f"ocs{h2}", name="ocs")
                nc.vector.tensor_copy(t_[:], pending_norm[2][h2][ds(0, HD), :])
                oc_sb.append(t_)
            p_, qq_ = pending_norm[0], pending_norm[1]
            nc.vector.tensor_mul(
                on_sb[p_][qq_][ds(0, HD), :], oc_sb[0][:], rbs[0][:]
            )
            om_ = smallp.tile([64, 512], F16, tag="om", name="om")
            nc.vector.tensor_mul(om_[:], oc_sb[1][:], rbs[1][:])
            nc.sync.dma_start(on_sb[p_][qq_][ds(64, HD), :], om_[:])

        # --- phase 3: output projection tail ---
        # Each head-pair's two heads are stacked on partitions 0-127 of
        # on_sb[p] / wo_sb[p], so one K=128 matmul per pair contracts over
        # both heads at once. PSUM->SBUF copies ride mostly on ACT (the DVE
        # is finishing the last round's norm), the store DMAs are fp16.
        with tc.tile_pool(name="opps", bufs=4, space="PSUM") as pD:
            for qt in range(LT):
                out_t = outp.tile([128, D], F16, tag="ot", name="out_t")
                psA = pD.tile([128, 1024], F32, tag="opA", name="psA")
                for oc in range(2):
                    for p in range(2):
                        nc.tensor.matmul(
                            psA[:, ts(oc, 512)],
                            lhsT=on_sb[p][qt // 4][:, ts(qt % 4, 128)],
                            rhs=wo_sb[p][:, ts(oc, 512)],
                            start=(p == 0),
                            stop=(p == 1),
                        )
                    if oc == 0 and qt >= 6:
                        nc.vector.tensor_copy(out_t[:, ts(oc, 512)], psA[:, ts(oc, 512)])
                    else:
                        nc.scalar.copy(out_t[:, ts(oc, 512)], psA[:, ts(oc, 512)])
                nc.sync.dma_start(out[ts(qt, 128), :], out_t[:])


def get_program():
    global _PROGRAM
    if _PROGRAM is None:
        _PROGRAM = _build_program()
    return _PROGRAM


def prepare_in_maps(q, k, v, Wq, bq, Wk, bk, Wv, bv, Wo, bo):
    """Build the 8 per-core input dicts (host-side slicing/transposes)."""
    q = np.asarray(q, dtype=np.float32)
    k = np.asarray(k, dtype=np.float32)
    v = np.asarray(v, dtype=np.float32)
    xT = {}
    for b in range(B):
        xT[("q", b)] = np.ascontiguousarray(q[b].T).astype(np.float16)
        xT[("k", b)] = np.ascontiguousarray(k[b].T).astype(np.float16)
        xT[("v", b)] = np.ascontiguousarray(v[b].T).astype(np.float16)
    ones_v = np.ones((128, LT, 4), dtype=np.float16)
    ones_r = np.ones((1, 64), dtype=np.float16)
    ident_m = np.eye(128, dtype=np.float16)
    in_maps = []
    for c in range(N_CORES):
        hg, b = c // 2, c % 2
        hs = hg * HS
        bq_s = np.asarray(bq, np.float32)[hs : hs + HS]
        bk_s = np.asarray(bk, np.float32)[hs : hs + HS]
        bv_s = np.asarray(bv, np.float32)[hs : hs + HS]
        bqkv_m = np.stack(
            [
                bq_s[0:128],
                bq_s[128:256],
                bk_s[0:128],
                bk_s[128:256],
                bv_s[0:128],
                bv_s[128:256],
            ],
            axis=1,
        )
        in_maps.append(
            {
                "xqT": xT[("q", b)],
                "xkT": xT[("k", b)],
                "xvT": xT[("v", b)],
                "wqT": np.asarray(Wq, np.float32)[hs : hs + HS, :].T.astype(np.float16),
                "wkT": np.asarray(Wk, np.float32)[hs : hs + HS, :].T.astype(np.float16),
                "wvT": np.asarray(Wv, np.float32)[hs : hs + HS, :].T.astype(np.float16),
                "woT": np.asarray(Wo, np.float32)[:, hs : hs + HS].T.astype(np.float16),
                "bqkv": np.ascontiguousarray(bqkv_m),
                "onesv": ones_v,
                "onesr": ones_r,
                "ident": ident_m,
            }
        )
    return in_maps


def combine_outputs(results, bo):
    """Sum head-group partials per batch and add the output bias."""
    bo = np.asarray(bo, np.float32)
    full = np.zeros((B, L, D), dtype=np.float32)
    for c in range(N_CORES):
        hg, b = c // 2, c % 2
        full[b] += np.asarray(results[c]["out"], dtype=np.float32)
    full += bo
    return full


def run(inputs, trace=False, trace_cores=None):
    nc = get_program()
    in_maps = prepare_in_maps(**inputs)
    res = run_bass_kernel_spmd(
        nc,
        in_maps,
        core_ids=list(range(N_CORES)),
        trace=trace,
        trace_cores=trace_cores,
    )
    out = combine_outputs(res.results, inputs["bo"])
    return out, res


def kernel(**inputs):
    out, _ = run(inputs, trace=False)
    return out



# revision 43
# speedup vs baseline: 1.2352x; 1.2308x over previous
"""Trainium2 Bass kernel for 16-head MHA (B=2, L=2048, D=1024), 8 NeuronCores.

Sharding: 8 cores = 4 head-groups x 2 batches. Core c handles head group
hg = c // 2 (4 heads = 256 of the 1024 projection columns) for batch
b = c % 2. Host sums the 4 head-group partials per batch and adds bo.

Per-core structure (v2): attention runs as 16 rounds of (256-query
block x head-pair), 16 key tiles per round.

- Scores: q is stored zero-PADDED ([h0;0] then [0;h1] per 256-query
  block), so ONE K=128 matmul per key tile streams both heads (the
  zero rows kill the cross terms).  One weight set per tile instead of
  two quadrant sets keeps the PE's double-buffered weight path fully
  pipelined (the quadrant-pair variant stalled ~210ns/tile waiting for
  a weight buffer), and score tiles are single PSUM banks (bufs=6), so
  the exp path has 5 tiles of slack and never throttles the PE.
- exp: ACT AF.Exp on 9/16 key tiles, Schraudolph int16 bit-trick on the
  DVE for the other 7 (error renormalizes away in softmax; ~7.5e-3
  total vs 2e-2 budget).
- P@V runs a full round behind scores; each head block of the
  augmented V is [64 dims | 64 ones-columns], so every PV matmul also
  broadcasts its softmax row sums into PSUM rows 64-127 for free
  (PV cost is stream-bound, M does not matter).  Both heads accumulate
  into one shared PSUM bank (h0 cols 0-255, h1 cols 256-511; h1 rides
  h0's start=True has_written region mark).
- Normalization is 4 DVE ops with no DMAs: tracked copy of the sum
  rows (PSUM->SBUF, orders against the PV stop), in-place
  reciprocal_approx_fast (free-size-bound: [64,512] costs the same as
  [1,512]; it cannot read PSUM directly - garbage on HW), then two
  tensor_muls writing the normalized halves (the h1 mul writes
  partitions 64-127 directly - engines can write a different partition
  base than they read).
- The q/k/v projections are chunk-streamed (x chunk = [128, KT, 512]);
  k-m1 fills round 0's empty PV slots, the v projection and the PE
  transposes of V fill round 1's.  The output projection reuses freed
  score PSUM banks and drains during the PV-only tail rounds.
"""
import sys

sys.path.insert(0, "/opt/trn_rl_repo")

import numpy as np

import concourse.bass as bass  # noqa: F401  (registers types)
import concourse.mybir as mybir
import concourse.tile as tile
from concourse import bacc
from concourse.bass import ds, ts
from concourse.bass_utils import run_bass_kernel_spmd

F32 = mybir.dt.float32
F16 = mybir.dt.float16
I16 = mybir.dt.int16
BF16 = mybir.dt.bfloat16
AF = mybir.ActivationFunctionType
ALU = mybir.AluOpType

# Schraudolph exp on DVE: fp16 bitpattern y = round(A*raw_score + B) so that
# int16->fp16 bitcast approximates exp(0.125*s) within +-3%. Applied to half
# the key tiles to offload the ACT engine (softmax renormalizes the shared
# bias; residual error ~8e-3 at 8/16 tiles, tolerance 2e-2).
EXP_A = 1024.0 / 0.6931471805599453 * 0.125
EXP_B = 15360.0 - 58.0
DVE_EXP_TILES = frozenset((1, 3, 5, 7, 9, 11, 13))

D = 1024          # model dim
L = 2048          # sequence length
B = 2             # batch
NH = 16           # total heads
HD = 64           # head dim
HS = 256          # head-slice columns per core (4 heads)
HC = 2 * HD       # head block: [64 dims | 64 ones] (ones-cols make the
                  # PV matmul broadcast softmax row sums to rows 64-127)
KT = D // 128     # 8 contraction tiles for projections
LT = L // 128     # 16 key tiles
QB = 256          # queries per round
NROUNDS = L // QB * 2   # 16 (query-block x head-pair)
N_CORES = 8

_PROGRAM = None


def _build_program():
    nc = bacc.Bacc(
        "TRN2",
        target_bir_lowering=False,
        debug=False,
        enable_asserts=False,
        num_devices=N_CORES,
    )
    xqT = nc.dram_tensor("xqT", (D, L), F16, kind="ExternalInput").ap()
    xkT = nc.dram_tensor("xkT", (D, L), F16, kind="ExternalInput").ap()
    xvT = nc.dram_tensor("xvT", (D, L), F16, kind="ExternalInput").ap()
    wqT = nc.dram_tensor("wqT", (D, HS), F16, kind="ExternalInput").ap()
    wkT = nc.dram_tensor("wkT", (D, HS), F16, kind="ExternalInput").ap()
    wvT = nc.dram_tensor("wvT", (D, HS), F16, kind="ExternalInput").ap()
    woT = nc.dram_tensor("woT", (HS, D), F16, kind="ExternalInput").ap()
    bqkv = nc.dram_tensor("bqkv", (128, 6), F32, kind="ExternalInput").ap()
    onesv = nc.dram_tensor("onesv", (128, LT, 4, HD), F16, kind="ExternalInput").ap()
    ident = nc.dram_tensor("ident", (128, 128), F16, kind="ExternalInput").ap()
    out = nc.dram_tensor("out", (L, D), F16, kind="ExternalOutput").ap()

    with tile.TileContext(nc) as tc:
        _emit(nc, tc, xqT, xkT, xvT, wqT, wkT, wvT, woT, bqkv, onesv, ident, out)
    nc.compile()
    return nc


def _emit(nc, tc, xqT, xkT, xvT, wqT, wkT, wvT, woT, bqkv, onesv, ident, out):
    with (
        tc.tile_pool(name="const", bufs=1) as constp,
        tc.tile_pool(name="wpool", bufs=1) as wpool,
        tc.tile_pool(name="proj", bufs=1) as projp,
        tc.tile_pool(name="xt", bufs=5) as xtp,
        tc.tile_pool(name="pt", bufs=36) as ptp,
        tc.tile_pool(name="norm", bufs=2) as normp,
        tc.tile_pool(name="outsb", bufs=3) as outp,
        tc.tile_pool(name="psum_s", bufs=6, space="PSUM") as pps,
        tc.tile_pool(name="psum_o", bufs=2, space="PSUM") as ppo,
    ):
        # --- constants (DMAs emitted later, after the first x chunks) ---
        bqkv_sb = constp.tile([128, 6], F32)
        ident_sb = constp.tile([128, 128], F16)

        # --- persistent activations ---
        qh_sb = [projp.tile([128, 2 * L], F16, tag=f"qh{m}", name=f"qh{m}") for m in range(2)]
        kh_sb = [projp.tile([128, L], F16, tag=f"kh{m}", name=f"kh{m}") for m in range(2)]
        vt_sb = [projp.tile([128, L], F16, tag=f"vt{m}", name=f"vt{m}") for m in range(2)]
        # augmented V: per key tile, 4 head blocks of [64 dims | ones col]
        vh_sb = projp.tile([128, LT, 4 * HC], F16, tag="vh", name="vh")
        vh4 = vh_sb[:].rearrange("p t (h c) -> p t h c", c=HC)
        on_sb = [projp.tile([128, L], F16, tag=f"on{p}", name=f"on{p}") for p in range(2)]

        def load_w(name, src, fine=False):
            t = wpool.tile([128, KT, HS], F16, tag=name, name=name)
            r = src.rearrange("(t p) c -> p t c", p=128)
            if fine:
                nc.sync.dma_start(t[:, ds(0, 1), :], r[:, ds(0, 1), :])
                nc.sync.dma_start(t[:, ds(1, 3), :], r[:, ds(1, 3), :])
            else:
                nc.sync.dma_start(t[:, ds(0, 4), :], r[:, ds(0, 4), :])
            nc.sync.dma_start(t[:, ds(4, 4), :], r[:, ds(4, 4), :])
            return t

        wq_r = wqT.rearrange("(t p) c -> p t c", p=128)
        xq_r = xqT.rearrange("(t p) c -> p t c", p=128)
        wq_sb = wpool.tile([128, KT, HS], F16, tag="wq", name="wq")
        xq_c0 = xtp.tile([128, KT, 512], F16, tag="xt")
        nc.sync.dma_start(wq_sb[:, ds(0, 1), :], wq_r[:, ds(0, 1), :])
        nc.sync.dma_start(xq_c0[:, ds(0, 1), :], xq_r[:, ds(0, 1), ds(0, 512)])
        nc.sync.dma_start(wq_sb[:, ds(1, 3), :], wq_r[:, ds(1, 3), :])
        nc.sync.dma_start(xq_c0[:, ds(1, 3), :], xq_r[:, ds(1, 3), ds(0, 512)])
        nc.sync.dma_start(wq_sb[:, ds(4, 4), :], wq_r[:, ds(4, 4), :])
        nc.sync.dma_start(xq_c0[:, ds(4, 4), :], xq_r[:, ds(4, 4), ds(0, 512)])
        nc.vector.memset(qh_sb[0][:], 0.0)
        nc.vector.memset(qh_sb[1][:], 0.0)
        # ones half of vh via DVE memset (a strided 1 MB DMA costs ~7 us;
        # the DVE is idle during the projections)
        nc.vector.memset(vh4[:, :, :, ds(HD, HD)], 1.0)

        # --- chunked projection helper ---
        # x chunk tile: [128, KT, 512] (all k-tiles of one 512-seq chunk)
        def x_chunk_dma(xdram, c, fine=False):
            xt_ = xtp.tile([128, KT, 512], F16, tag="xt")
            src = xdram.rearrange("(t p) c -> p t c", p=128)
            if fine:
                nc.sync.dma_start(xt_[:, ds(0, 1), :], src[:, ds(0, 1), ts(c, 512)])
                nc.sync.dma_start(xt_[:, ds(1, 3), :], src[:, ds(1, 3), ts(c, 512)])
            else:
                nc.sync.dma_start(xt_[:, ds(0, 4), :], src[:, ds(0, 4), ts(c, 512)])
            nc.sync.dma_start(xt_[:, ds(4, 4), :], src[:, ds(4, 4), ts(c, 512)])
            return xt_

        def proj_chunk(w_sb, xt_, dst, bias0, m, c, qpad=False):
            """One (m, chunk) projection group: 8 accum MMs + bias ACT."""
            ps = ppo.tile([128, 512], F32, tag="o", name="pj")
            for t in range(KT):
                nc.tensor.matmul(
                    ps[:],
                    lhsT=w_sb[:, t, ts(m, 128)],
                    rhs=xt_[:, t, :],
                    start=(t == 0),
                    stop=(t == KT - 1),
                )
            if qpad:
                # padded layout: h0 rows -> even 512-block, h1 rows -> odd,
                # so one K=128 scores matmul streams both heads (the zero
                # half of each column contributes nothing)
                nc.scalar.activation(
                    dst[ds(0, 64), ds(c * 1024, 512)], ps[ds(0, 64), :],
                    AF.Identity, bias=bqkv_sb[ds(0, 64), ds(bias0 + m, 1)],
                )
                nc.scalar.activation(
                    dst[ds(64, 64), ds(c * 1024 + 512, 512)], ps[ds(64, 64), :],
                    AF.Identity, bias=bqkv_sb[ds(64, 64), ds(bias0 + m, 1)],
                )
            else:
                nc.scalar.activation(
                    dst[:, ts(c, 512)], ps[:], AF.Identity,
                    bias=bqkv_sb[:, ds(bias0 + m, 1)],
                )

        # --- P0: q-proj (m0, m1) + k-proj m0 ---
        xq_chunks = [xq_c0]
        nc.sync.dma_start(bqkv_sb[:], bqkv)
        xq_chunks += [x_chunk_dma(xqT, c) for c in range(1, 4)]
        nc.sync.dma_start(ident_sb[:], ident)
        wk_sb = load_w("wk", wkT)
        for c in range(4):
            for m in range(2):
                proj_chunk(wq_sb, xq_chunks[c], qh_sb[m], 0, m, c, qpad=True)
        xk_chunks = [x_chunk_dma(xkT, c) for c in range(4)]
        wv_sb = load_w("wv", wvT)
        wo_sb = []
        for p in range(2):
            t = wpool.tile([128, D], F16, tag=f"wo{p}", name=f"wo{p}")
            nc.sync.dma_start(t[:], woT[ts(p, 128), :])
            wo_sb.append(t)
        for c in range(4):
            proj_chunk(wk_sb, xk_chunks[c], kh_sb[0], 2, 0, c)

        # --- attention rounds ---
        # round r: qblk = r//2, pair p = r%2
        #   scores(r, t) + exp -> p_t ; PV(r-2, t) ; norm(r-3) ; outproj
        # rounds 0-2 PV slots carry fillers:
        #   r0: k-proj m1 (4 chunks) + xv prefetch
        #   r1: v-proj m0+m1 (8 chunks) + transposes p0
        #   r2: transposes p1
        xv_chunks = [None] * 4
        o_tiles = {}       # round j -> o PSUM tile
        p_tiles = {}       # (round j, t) -> p_t SBUF tile
        norm_state = {}

        # padded-q view: [(chunk, qb256)][half][256] -> one K=128 scores
        # matmul per key tile streams both heads (zero rows kill the
        # cross terms), so the pair needs ONE weight set, not two.
        qview = [
            qh_sb[p][:].rearrange("p (c half q) -> p c half q", half=2, q=512)
            for p in range(2)
        ]

        def emit_scores_exp(r, t):
            p = r % 2
            qblk = r // 2
            c_idx, off = (qblk * QB) // 512, (qblk * QB) % 512
            s_sl = pps.tile([128, 512], F32, tag="s", name="s")
            nc.tensor.matmul(
                s_sl[:].rearrange("p (h c) -> p h c", c=256),
                lhsT=kh_sb[p][:, ts(t, 128)],
                rhs=qview[p][:, c_idx, :, ds(off, QB)],
                start=True,
                stop=True,
            )
            p_t = ptp.tile([128, 512], F16, tag="pt", name="p_t")
            if t in DVE_EXP_TILES:
                nc.vector.tensor_scalar(
                    p_t[:].bitcast(I16), s_sl[:], EXP_A, EXP_B,
                    ALU.mult, ALU.add,
                )
            else:
                nc.scalar.activation(p_t[:], s_sl[:], AF.Exp, scale=0.125)
            p_tiles[(r, t)] = p_t

        def emit_pv(j, t):
            """P@V for round j, key tile t (both heads into one bank)."""
            pj = j % 2
            if t == 0:
                o_tiles[j] = ppo.tile([128, 512], F32, tag="o", name="o")
            o_t = o_tiles[j]
            p_t = p_tiles.pop((j, t))
            for h2 in range(2):
                nc.tensor.matmul(
                    o_t[:, ts(h2, QB)],
                    lhsT=vh4[:, t, 2 * pj + h2, :],
                    rhs=p_t[:, ts(h2, QB)],
                    start=(t == 0 and h2 == 0),
                    stop=(t == LT - 1),
                    skip_group_check=True,
                )

        def emit_norm(j, step):
            """Normalize round j: all-DVE, no DMA.

            The PV ones-columns replicated the row sums into o rows
            64-127, so the reciprocal runs at full lane width straight
            out of PSUM (reciprocal_approx_fast is free-size-bound).
            The tiny tracked copy before it fences the PE stop; the
            mults that consume it are same-engine so no fence after.
            """
            pj, qbj = j % 2, j // 2
            if step == 0:
                ssum = normp.tile([64, 512], F32, tag="ssum", name="ssum")
                sraw = normp.tile([64, 512], F32, tag="sraw", name="sraw")
                norm_state[j] = (ssum, sraw)
                # tracked PSUM->SBUF copies (split across ACT and DVE for
                # balance) order against the PV stop; the custom recip
                # then runs SBUF->SBUF behind the DVE half (same engine)
                nc.scalar.copy(sraw[:, ds(0, QB)], o_tiles[j][ds(64, 64), ds(0, QB)])
                nc.vector.tensor_copy(
                    sraw[:, ds(QB, QB)], o_tiles[j][ds(64, 64), ds(QB, QB)]
                )
                nc.vector.tensor_copy(ssum[ds(0, 1), ds(0, 8)], sraw[ds(0, 1), ds(0, 8)])
                nc.vector.reciprocal_approx_fast(ssum[:], sraw[:])
            elif step == 1:
                ssum, sraw = norm_state[j]
                nc.vector.tensor_mul(
                    on_sb[pj][ds(0, HD), ds(qbj * QB, QB)],
                    o_tiles[j][ds(0, HD), ds(0, QB)], ssum[:, ds(0, QB)],
                )
            elif step == 2:
                ssum, sraw = norm_state[j]
                nc.vector.tensor_mul(
                    on_sb[pj][ds(64, HD), ds(qbj * QB, QB)],
                    o_tiles[j][ds(0, HD), ds(QB, QB)], ssum[:, ds(QB, QB)],
                )
                o_tiles.pop(j)
                norm_state.pop(j)

        def emit_outproj(qt, act_only=False):
            out_t = outp.tile([128, D], F16, tag="ot", name="out_t")
            for oc_ in range(2):
                psA = pps.tile([128, 512], F32, tag="s", name="psA")
                for p2 in range(2):
                    nc.tensor.matmul(
                        psA[:],
                        lhsT=on_sb[p2][:, ts(qt, 128)],
                        rhs=wo_sb[p2][:, ts(oc_, 512)],
                        start=(p2 == 0),
                        stop=(p2 == 1),
                    )
                if oc_ == 1 and not act_only:
                    nc.vector.tensor_copy(out_t[:, ts(oc_, 512)], psA[:])
                else:
                    nc.scalar.copy(out_t[:, ts(oc_, 512)], psA[:])
            nc.sync.dma_start(out[ts(qt, 128), :], out_t[:])

        def emit_transpose_group(p, g):
            """8 PE-transposes of vt[p] key tiles g*8..g*8+7 into vh."""
            trps = ppo.tile([128, 512], F32, tag="o", name="trps")
            slots = []
            for i in range(8):
                m = g * 8 + i
                slot = trps[:, ts(i, 64)].bitcast(F16)
                nc.tensor.transpose(slot, vt_sb[p][:, ts(m, 128)], ident_sb[:])
                slots.append((m, slot))
            for m, slot in slots:
                nc.vector.tensor_copy(
                    vh4[:, m, ds(2 * p, 2), ds(0, HD)],
                    slot.rearrange("p (h c) -> p h c", c=HD),
                )

        def fillers_for_round(r):
            f = {}
            if r == 0:
                def mk(c):
                    def go():
                        proj_chunk(wk_sb, xk_chunks[c], kh_sb[1], 2, 1, c)
                    return go
                for i, c in enumerate(range(4)):
                    f[2 + 3 * i] = mk(c)

                def mkxv(c):
                    def go():
                        xv_chunks[c] = x_chunk_dma(xvT, c)
                    return go
                f[0] = mkxv(0)
                f[1] = mkxv(1)
                f[13] = mkxv(2)
                f[14] = mkxv(3)
            elif r == 1:
                def mkv(m, c):
                    def go():
                        proj_chunk(wv_sb, xv_chunks[c], vt_sb[m], 4, m, c)
                    return go
                for c in range(4):
                    f[2 * c] = mkv(0, c)
                f[8] = lambda: emit_transpose_group(0, 0)
                f[9] = lambda: emit_transpose_group(0, 1)
                for c in range(4):
                    f[10 + c] = mkv(1, c)
                f[14] = lambda: emit_transpose_group(1, 0)
                f[15] = lambda: emit_transpose_group(1, 1)
            return f

        # outproj schedule: qt needs norms of rounds qt and qt^1 done.
        # norm(j) runs during round j+3 and its recip/broadcast chain is
        # ~11us of DMA+engine latency; emitting outproj at round 2b+7
        # keeps that chain far off the PE's in-order critical path.
        outproj_sched = {}
        for b in range(6):
            outproj_sched.setdefault(2 * b + 4, []).append((8, 2 * b))
            outproj_sched.setdefault(2 * b + 5, []).append((8, 2 * b + 1))

        for r in range(NROUNDS):
            fill = fillers_for_round(r)
            ops = {t: [] for t in range(LT)}
            for t_slot, qt in outproj_sched.get(r, []):
                ops[t_slot].append(qt)
            for t in range(LT):
                if r >= 2:
                    emit_pv(r - 2, t)
                emit_scores_exp(r, t)
                if t in fill:
                    fill[t]()
                if r >= 3 and 1 <= t <= 3:
                    emit_norm(r - 3, t - 1)
                for qt in ops[t]:
                    emit_outproj(qt)

        # --- drain: PV(14), PV(15), final norms, last outproj tiles ---
        for t in range(LT):
            emit_pv(14, t)
            if 1 <= t <= 3:
                emit_norm(13, t - 1)
            if t == 6:
                emit_outproj(12)
        for t in range(LT):
            emit_pv(15, t)
            if 1 <= t <= 3:
                emit_norm(14, t - 1)
            if t == 6:
                emit_outproj(13, act_only=True)
        for step in range(3):
            emit_norm(15, step)
        emit_outproj(13, act_only=True)
        emit_outproj(14)
        emit_outproj(15)


def get_program():
    global _PROGRAM
    if _PROGRAM is None:
        _PROGRAM = _build_program()
    return _PROGRAM


def prepare_in_maps(q, k, v, Wq, bq, Wk, bk, Wv, bv, Wo, bo):
    """Build the 8 per-core input dicts (host-side slicing/transposes)."""
    q = np.asarray(q, dtype=np.float32)
    k = np.asarray(k, dtype=np.float32)
    v = np.asarray(v, dtype=np.float32)
    xT = {}
    for b in range(B):
        xT[("q", b)] = np.ascontiguousarray(q[b].T).astype(np.float16)
        xT[("k", b)] = np.ascontiguousarray(k[b].T).astype(np.float16)
        xT[("v", b)] = np.ascontiguousarray(v[b].T).astype(np.float16)
    ones_v = np.ones((128, LT, 4, HD), dtype=np.float16)
    ident_m = np.eye(128, dtype=np.float16)
    in_maps = []
    for c in range(N_CORES):
        hg, b = c // 2, c % 2
        hs = hg * HS
        bq_s = np.asarray(bq, np.float32)[hs : hs + HS]
        bk_s = np.asarray(bk, np.float32)[hs : hs + HS]
        bv_s = np.asarray(bv, np.float32)[hs : hs + HS]
        bqkv_m = np.stack(
            [
                bq_s[0:128],
                bq_s[128:256],
                bk_s[0:128],
                bk_s[128:256],
                bv_s[0:128],
                bv_s[128:256],
            ],
            axis=1,
        )
        in_maps.append(
            {
                "xqT": xT[("q", b)],
                "xkT": xT[("k", b)],
                "xvT": xT[("v", b)],
                "wqT": np.asarray(Wq, np.float32)[hs : hs + HS, :].T.astype(np.float16),
                "wkT": np.asarray(Wk, np.float32)[hs : hs + HS, :].T.astype(np.float16),
                "wvT": np.asarray(Wv, np.float32)[hs : hs + HS, :].T.astype(np.float16),
                "woT": np.asarray(Wo, np.float32)[:, hs : hs + HS].T.astype(np.float16),
                "bqkv": np.ascontiguousarray(bqkv_m),
                "onesv": ones_v,
                "ident": ident_m,
            }
        )
    return in_maps


def combine_outputs(results, bo):
    """Sum head-group partials per batch and add the output bias."""
    bo = np.asarray(bo, np.float32)
    full = np.zeros((B, L, D), dtype=np.float32)
    for c in range(N_CORES):
        hg, b = c // 2, c % 2
        full[b] += np.asarray(results[c]["out"], dtype=np.float32)
    full += bo
    return full


def run(inputs, trace=False, trace_cores=None):
    nc = get_program()
    in_maps = prepare_in_maps(**inputs)
    res = run_bass_kernel_spmd(
        nc,
        in_maps,
        core_ids=list(range(N_CORES)),
        trace=trace,
        trace_cores=trace_cores,
    )
    out = combine_outputs(res.results, inputs["bo"])
    return out, res


def kernel(**inputs):
    out, _ = run(inputs, trace=False)
    return out


# revision 44
# speedup vs baseline: 1.2376x; 1.0019x over previous
"""Trainium2 Bass kernel for 16-head MHA (B=2, L=2048, D=1024), 8 NeuronCores.

Sharding: 8 cores = 4 head-groups x 2 batches. Core c handles head group
hg = c // 2 (4 heads = 256 of the 1024 projection columns) for batch
b = c % 2. Host sums the 4 head-group partials per batch and adds bo.

Per-core structure (v2): attention runs as 16 rounds of (256-query
block x head-pair), 16 key tiles per round.

- Scores: q is stored zero-PADDED ([h0;0] then [0;h1] per 256-query
  block), so ONE K=128 matmul per key tile streams both heads (the
  zero rows kill the cross terms).  One weight set per tile instead of
  two quadrant sets keeps the PE's double-buffered weight path fully
  pipelined (the quadrant-pair variant stalled ~210ns/tile waiting for
  a weight buffer), and score tiles are single PSUM banks (bufs=6), so
  the exp path has 5 tiles of slack and never throttles the PE.
- exp: ACT AF.Exp on 9/16 key tiles, Schraudolph int16 bit-trick on the
  DVE for the other 7 (error renormalizes away in softmax; ~7.5e-3
  total vs 2e-2 budget).
- P@V runs a full round behind scores; each head block of the
  augmented V is [64 dims | 64 ones-columns], so every PV matmul also
  broadcasts its softmax row sums into PSUM rows 64-127 for free
  (PV cost is stream-bound, M does not matter).  Both heads accumulate
  into one shared PSUM bank (h0 cols 0-255, h1 cols 256-511; h1 rides
  h0's start=True has_written region mark).
- Normalization is 4 DVE ops with no DMAs: tracked copy of the sum
  rows (PSUM->SBUF, orders against the PV stop), in-place
  reciprocal_approx_fast (free-size-bound: [64,512] costs the same as
  [1,512]; it cannot read PSUM directly - garbage on HW), then two
  tensor_muls writing the normalized halves (the h1 mul writes
  partitions 64-127 directly - engines can write a different partition
  base than they read).
- The q/k/v projections are chunk-streamed (x chunk = [128, KT, 512]);
  k-m1 fills round 0's empty PV slots, the v projection and the PE
  transposes of V fill round 1's.  The output projection reuses freed
  score PSUM banks and drains during the PV-only tail rounds.
"""
import sys

sys.path.insert(0, "/opt/trn_rl_repo")

import numpy as np

import concourse.bass as bass  # noqa: F401  (registers types)
import concourse.mybir as mybir
import concourse.tile as tile
from concourse import bacc
from concourse.bass import ds, ts
from concourse.bass_utils import run_bass_kernel_spmd

F32 = mybir.dt.float32
F16 = mybir.dt.float16
I16 = mybir.dt.int16
BF16 = mybir.dt.bfloat16
AF = mybir.ActivationFunctionType
ALU = mybir.AluOpType

# Schraudolph exp on DVE: fp16 bitpattern y = round(A*raw_score + B) so that
# int16->fp16 bitcast approximates exp(0.125*s) within +-3%. Applied to half
# the key tiles to offload the ACT engine (softmax renormalizes the shared
# bias; residual error ~8e-3 at 8/16 tiles, tolerance 2e-2).
EXP_A = 1024.0 / 0.6931471805599453 * 0.125
EXP_B = 15360.0 - 58.0
DVE_EXP_TILES = frozenset((1, 3, 5, 7, 9, 11, 13))

D = 1024          # model dim
L = 2048          # sequence length
B = 2             # batch
NH = 16           # total heads
HD = 64           # head dim
HS = 256          # head-slice columns per core (4 heads)
HC = 2 * HD       # head block: [64 dims | 64 ones] (ones-cols make the
                  # PV matmul broadcast softmax row sums to rows 64-127)
KT = D // 128     # 8 contraction tiles for projections
LT = L // 128     # 16 key tiles
QB = 256          # queries per round
NROUNDS = L // QB * 2   # 16 (query-block x head-pair)
N_CORES = 8

_PROGRAM = None


def _build_program():
    nc = bacc.Bacc(
        "TRN2",
        target_bir_lowering=False,
        debug=False,
        enable_asserts=False,
        num_devices=N_CORES,
    )
    xqT = nc.dram_tensor("xqT", (D, L), F16, kind="ExternalInput").ap()
    xkT = nc.dram_tensor("xkT", (D, L), F16, kind="ExternalInput").ap()
    xvT = nc.dram_tensor("xvT", (D, L), F16, kind="ExternalInput").ap()
    wqT = nc.dram_tensor("wqT", (D, HS), F16, kind="ExternalInput").ap()
    wkT = nc.dram_tensor("wkT", (D, HS), F16, kind="ExternalInput").ap()
    wvT = nc.dram_tensor("wvT", (D, HS), F16, kind="ExternalInput").ap()
    woT = nc.dram_tensor("woT", (HS, D), F16, kind="ExternalInput").ap()
    bqkv = nc.dram_tensor("bqkv", (128, 6), F32, kind="ExternalInput").ap()
    onesv = nc.dram_tensor("onesv", (128, LT, 4, HD), F16, kind="ExternalInput").ap()
    ident = nc.dram_tensor("ident", (128, 128), F16, kind="ExternalInput").ap()
    out = nc.dram_tensor("out", (L, D), F16, kind="ExternalOutput").ap()

    with tile.TileContext(nc) as tc:
        _emit(nc, tc, xqT, xkT, xvT, wqT, wkT, wvT, woT, bqkv, onesv, ident, out)
    nc.compile()
    return nc


def _emit(nc, tc, xqT, xkT, xvT, wqT, wkT, wvT, woT, bqkv, onesv, ident, out):
    with (
        tc.tile_pool(name="const", bufs=1) as constp,
        tc.tile_pool(name="wpool", bufs=1) as wpool,
        tc.tile_pool(name="proj", bufs=1) as projp,
        tc.tile_pool(name="xt", bufs=5) as xtp,
        tc.tile_pool(name="pt", bufs=36) as ptp,
        tc.tile_pool(name="norm", bufs=2) as normp,
        tc.tile_pool(name="outsb", bufs=3) as outp,
        tc.tile_pool(name="psum_s", bufs=6, space="PSUM") as pps,
        tc.tile_pool(name="psum_o", bufs=2, space="PSUM") as ppo,
    ):
        # --- constants (DMAs emitted later, after the first x chunks) ---
        bqkv_sb = constp.tile([128, 6], F32)
        ident_sb = constp.tile([128, 128], F16)

        # --- persistent activations ---
        qh_sb = [projp.tile([128, 2 * L], F16, tag=f"qh{m}", name=f"qh{m}") for m in range(2)]
        kh_sb = [projp.tile([128, L], F16, tag=f"kh{m}", name=f"kh{m}") for m in range(2)]
        vt_sb = [projp.tile([128, L], F16, tag=f"vt{m}", name=f"vt{m}") for m in range(2)]
        # augmented V: per key tile, 4 head blocks of [64 dims | ones col]
        vh_sb = projp.tile([128, LT, 4 * HC], F16, tag="vh", name="vh")
        vh4 = vh_sb[:].rearrange("p t (h c) -> p t h c", c=HC)
        on_sb = [projp.tile([128, L], F16, tag=f"on{p}", name=f"on{p}") for p in range(2)]

        # HAM warm-up: ~20 dummy matmuls on (uninitialized) SBUF into a
        # never-read scratch bank, emitted before any DMA.  They fill the
        # ~8us of init+DMA-ramp PE idle so the clock gate is at K=8/8
        # (2.4 GHz) when the first real projection matmul issues; they
        # finish before its input data lands, so they delay nothing.
        for w_ in range(20):
            warm_ps = pps.tile([128, 512], F32, tag="s", name="warm")
            nc.tensor.matmul(
                warm_ps[:],
                lhsT=on_sb[0][:, ds(0, 128)],
                rhs=on_sb[0][:, ds(0, 512)],
                start=True,
                stop=True,
            )

        def load_w(name, src, fine=False):
            t = wpool.tile([128, KT, HS], F16, tag=name, name=name)
            r = src.rearrange("(t p) c -> p t c", p=128)
            if fine:
                nc.sync.dma_start(t[:, ds(0, 1), :], r[:, ds(0, 1), :])
                nc.sync.dma_start(t[:, ds(1, 3), :], r[:, ds(1, 3), :])
            else:
                nc.sync.dma_start(t[:, ds(0, 4), :], r[:, ds(0, 4), :])
            nc.sync.dma_start(t[:, ds(4, 4), :], r[:, ds(4, 4), :])
            return t

        wq_r = wqT.rearrange("(t p) c -> p t c", p=128)
        xq_r = xqT.rearrange("(t p) c -> p t c", p=128)
        wq_sb = wpool.tile([128, KT, HS], F16, tag="wq", name="wq")
        xq_c0 = xtp.tile([128, KT, 512], F16, tag="xt")
        nc.sync.dma_start(wq_sb[:, ds(0, 1), :], wq_r[:, ds(0, 1), :])
        nc.sync.dma_start(xq_c0[:, ds(0, 1), :], xq_r[:, ds(0, 1), ds(0, 512)])
        nc.sync.dma_start(wq_sb[:, ds(1, 3), :], wq_r[:, ds(1, 3), :])
        nc.sync.dma_start(xq_c0[:, ds(1, 3), :], xq_r[:, ds(1, 3), ds(0, 512)])
        nc.sync.dma_start(wq_sb[:, ds(4, 4), :], wq_r[:, ds(4, 4), :])
        nc.sync.dma_start(xq_c0[:, ds(4, 4), :], xq_r[:, ds(4, 4), ds(0, 512)])
        nc.vector.memset(qh_sb[0][:], 0.0)
        nc.vector.memset(qh_sb[1][:], 0.0)
        # ones half of vh via DVE memset (a strided 1 MB DMA costs ~7 us;
        # the DVE is idle during the projections)
        nc.vector.memset(vh4[:, :, :, ds(HD, HD)], 1.0)

        # --- chunked projection helper ---
        # x chunk tile: [128, KT, 512] (all k-tiles of one 512-seq chunk)
        def x_chunk_dma(xdram, c, fine=False):
            xt_ = xtp.tile([128, KT, 512], F16, tag="xt")
            src = xdram.rearrange("(t p) c -> p t c", p=128)
            if fine:
                nc.sync.dma_start(xt_[:, ds(0, 1), :], src[:, ds(0, 1), ts(c, 512)])
                nc.sync.dma_start(xt_[:, ds(1, 3), :], src[:, ds(1, 3), ts(c, 512)])
            else:
                nc.sync.dma_start(xt_[:, ds(0, 4), :], src[:, ds(0, 4), ts(c, 512)])
            nc.sync.dma_start(xt_[:, ds(4, 4), :], src[:, ds(4, 4), ts(c, 512)])
            return xt_

        def proj_chunk(w_sb, xt_, dst, bias0, m, c, qpad=False):
            """One (m, chunk) projection group: 8 accum MMs + bias ACT."""
            ps = ppo.tile([128, 512], F32, tag="o", name="pj")
            for t in range(KT):
                nc.tensor.matmul(
                    ps[:],
                    lhsT=w_sb[:, t, ts(m, 128)],
                    rhs=xt_[:, t, :],
                    start=(t == 0),
                    stop=(t == KT - 1),
                )
            if qpad:
                # padded layout: h0 rows -> even 512-block, h1 rows -> odd,
                # so one K=128 scores matmul streams both heads (the zero
                # half of each column contributes nothing)
                nc.scalar.activation(
                    dst[ds(0, 64), ds(c * 1024, 512)], ps[ds(0, 64), :],
                    AF.Identity, bias=bqkv_sb[ds(0, 64), ds(bias0 + m, 1)],
                )
                nc.scalar.activation(
                    dst[ds(64, 64), ds(c * 1024 + 512, 512)], ps[ds(64, 64), :],
                    AF.Identity, bias=bqkv_sb[ds(64, 64), ds(bias0 + m, 1)],
                )
            else:
                nc.scalar.activation(
                    dst[:, ts(c, 512)], ps[:], AF.Identity,
                    bias=bqkv_sb[:, ds(bias0 + m, 1)],
                )

        # --- P0: q-proj (m0, m1) + k-proj m0 ---
        xq_chunks = [xq_c0]
        nc.sync.dma_start(bqkv_sb[:], bqkv)
        xq_chunks += [x_chunk_dma(xqT, c) for c in range(1, 4)]
        nc.sync.dma_start(ident_sb[:], ident)
        wk_sb = load_w("wk", wkT)
        for c in range(4):
            for m in range(2):
                proj_chunk(wq_sb, xq_chunks[c], qh_sb[m], 0, m, c, qpad=True)
        xk_chunks = [x_chunk_dma(xkT, c) for c in range(4)]
        wv_sb = load_w("wv", wvT)
        wo_sb = []
        for p in range(2):
            t = wpool.tile([128, D], F16, tag=f"wo{p}", name=f"wo{p}")
            nc.sync.dma_start(t[:], woT[ts(p, 128), :])
            wo_sb.append(t)
        for c in range(4):
            proj_chunk(wk_sb, xk_chunks[c], kh_sb[0], 2, 0, c)

        # --- attention rounds ---
        # round r: qblk = r//2, pair p = r%2
        #   scores(r, t) + exp -> p_t ; PV(r-2, t) ; norm(r-3) ; outproj
        # rounds 0-2 PV slots carry fillers:
        #   r0: k-proj m1 (4 chunks) + xv prefetch
        #   r1: v-proj m0+m1 (8 chunks) + transposes p0
        #   r2: transposes p1
        xv_chunks = [None] * 4
        o_tiles = {}       # round j -> o PSUM tile
        p_tiles = {}       # (round j, t) -> p_t SBUF tile
        norm_state = {}

        # padded-q view: [(chunk, qb256)][half][256] -> one K=128 scores
        # matmul per key tile streams both heads (zero rows kill the
        # cross terms), so the pair needs ONE weight set, not two.
        qview = [
            qh_sb[p][:].rearrange("p (c half q) -> p c half q", half=2, q=512)
            for p in range(2)
        ]

        def emit_scores_exp(r, t):
            p = r % 2
            qblk = r // 2
            c_idx, off = (qblk * QB) // 512, (qblk * QB) % 512
            s_sl = pps.tile([128, 512], F32, tag="s", name="s")
            nc.tensor.matmul(
                s_sl[:].rearrange("p (h c) -> p h c", c=256),
                lhsT=kh_sb[p][:, ts(t, 128)],
                rhs=qview[p][:, c_idx, :, ds(off, QB)],
                start=True,
                stop=True,
            )
            p_t = ptp.tile([128, 512], F16, tag="pt", name="p_t")
            if t in DVE_EXP_TILES:
                nc.vector.tensor_scalar(
                    p_t[:].bitcast(I16), s_sl[:], EXP_A, EXP_B,
                    ALU.mult, ALU.add,
                )
            else:
                nc.scalar.activation(p_t[:], s_sl[:], AF.Exp, scale=0.125)
            p_tiles[(r, t)] = p_t

        def emit_pv(j, t):
            """P@V for round j, key tile t (both heads into one bank)."""
            pj = j % 2
            if t == 0:
                o_tiles[j] = ppo.tile([128, 512], F32, tag="o", name="o")
            o_t = o_tiles[j]
            p_t = p_tiles.pop((j, t))
            for h2 in range(2):
                nc.tensor.matmul(
                    o_t[:, ts(h2, QB)],
                    lhsT=vh4[:, t, 2 * pj + h2, :],
                    rhs=p_t[:, ts(h2, QB)],
                    start=(t == 0 and h2 == 0),
                    stop=(t == LT - 1),
                    skip_group_check=True,
                )

        def emit_norm(j, step):
            """Normalize round j: all-DVE, no DMA.

            The PV ones-columns replicated the row sums into o rows
            64-127, so the reciprocal runs at full lane width straight
            out of PSUM (reciprocal_approx_fast is free-size-bound).
            The tiny tracked copy before it fences the PE stop; the
            mults that consume it are same-engine so no fence after.
            """
            pj, qbj = j % 2, j // 2
            if step == 0:
                ssum = normp.tile([64, 512], F32, tag="ssum", name="ssum")
                sraw = normp.tile([64, 512], F32, tag="sraw", name="sraw")
                norm_state[j] = (ssum, sraw)
                # tracked PSUM->SBUF copies (split across ACT and DVE for
                # balance) order against the PV stop; the custom recip
                # then runs SBUF->SBUF behind the DVE half (same engine)
                nc.scalar.copy(sraw[:, ds(0, QB)], o_tiles[j][ds(64, 64), ds(0, QB)])
                nc.vector.tensor_copy(
                    sraw[:, ds(QB, QB)], o_tiles[j][ds(64, 64), ds(QB, QB)]
                )
                nc.vector.tensor_copy(ssum[ds(0, 1), ds(0, 8)], sraw[ds(0, 1), ds(0, 8)])
                nc.vector.reciprocal_approx_fast(ssum[:], sraw[:])
            elif step == 1:
                ssum, sraw = norm_state[j]
                nc.vector.tensor_mul(
                    on_sb[pj][ds(0, HD), ds(qbj * QB, QB)],
                    o_tiles[j][ds(0, HD), ds(0, QB)], ssum[:, ds(0, QB)],
                )
            elif step == 2:
                ssum, sraw = norm_state[j]
                nc.vector.tensor_mul(
                    on_sb[pj][ds(64, HD), ds(qbj * QB, QB)],
                    o_tiles[j][ds(0, HD), ds(QB, QB)], ssum[:, ds(QB, QB)],
                )
                o_tiles.pop(j)
                norm_state.pop(j)

        def emit_outproj(qt, act_only=False):
            out_t = outp.tile([128, D], F16, tag="ot", name="out_t")
            for oc_ in range(2):
                psA = pps.tile([128, 512], F32, tag="s", name="psA")
                for p2 in range(2):
                    nc.tensor.matmul(
                        psA[:],
                        lhsT=on_sb[p2][:, ts(qt, 128)],
                        rhs=wo_sb[p2][:, ts(oc_, 512)],
                        start=(p2 == 0),
                        stop=(p2 == 1),
                    )
                if oc_ == 1 and not act_only:
                    nc.vector.tensor_copy(out_t[:, ts(oc_, 512)], psA[:])
                else:
                    nc.scalar.copy(out_t[:, ts(oc_, 512)], psA[:])
            nc.sync.dma_start(out[ts(qt, 128), :], out_t[:])

        def emit_transpose_group(p, g):
            """8 PE-transposes of vt[p] key tiles g*8..g*8+7 into vh."""
            trps = ppo.tile([128, 512], F32, tag="o", name="trps")
            slots = []
            for i in range(8):
                m = g * 8 + i
                slot = trps[:, ts(i, 64)].bitcast(F16)
                nc.tensor.transpose(slot, vt_sb[p][:, ts(m, 128)], ident_sb[:])
                slots.append((m, slot))
            for m, slot in slots:
                nc.vector.tensor_copy(
                    vh4[:, m, ds(2 * p, 2), ds(0, HD)],
                    slot.rearrange("p (h c) -> p h c", c=HD),
                )

        def fillers_for_round(r):
            f = {}
            if r == 0:
                def mk(c):
                    def go():
                        proj_chunk(wk_sb, xk_chunks[c], kh_sb[1], 2, 1, c)
                    return go
                for i, c in enumerate(range(4)):
                    f[2 + 3 * i] = mk(c)

                def mkxv(c):
                    def go():
                        xv_chunks[c] = x_chunk_dma(xvT, c)
                    return go
                f[0] = mkxv(0)
                f[1] = mkxv(1)
                f[13] = mkxv(2)
                f[14] = mkxv(3)
            elif r == 1:
                def mkv(m, c):
                    def go():
                        proj_chunk(wv_sb, xv_chunks[c], vt_sb[m], 4, m, c)
                    return go
                for c in range(4):
                    f[2 * c] = mkv(0, c)
                f[8] = lambda: emit_transpose_group(0, 0)
                f[9] = lambda: emit_transpose_group(0, 1)
                for c in range(4):
                    f[10 + c] = mkv(1, c)
                f[14] = lambda: emit_transpose_group(1, 0)
                f[15] = lambda: emit_transpose_group(1, 1)
            return f

        # outproj schedule: qt needs norms of rounds qt and qt^1 done.
        # norm(j) runs during round j+3 and its recip/broadcast chain is
        # ~11us of DMA+engine latency; emitting outproj at round 2b+7
        # keeps that chain far off the PE's in-order critical path.
        outproj_sched = {}
        for b in range(6):
            outproj_sched.setdefault(2 * b + 4, []).append((8, 2 * b))
            outproj_sched.setdefault(2 * b + 5, []).append((8, 2 * b + 1))

        for r in range(NROUNDS):
            fill = fillers_for_round(r)
            ops = {t: [] for t in range(LT)}
            for t_slot, qt in outproj_sched.get(r, []):
                ops[t_slot].append(qt)
            for t in range(LT):
                if r >= 2:
                    emit_pv(r - 2, t)
                emit_scores_exp(r, t)
                if t in fill:
                    fill[t]()
                if r >= 3 and 1 <= t <= 3:
                    emit_norm(r - 3, t - 1)
                for qt in ops[t]:
                    emit_outproj(qt)

        # --- drain: PV(14), PV(15), final norms, last outproj tiles ---
        for t in range(LT):
            emit_pv(14, t)
            if 1 <= t <= 3:
                emit_norm(13, t - 1)
            if t == 6:
                emit_outproj(12)
        for t in range(LT):
            emit_pv(15, t)
            if 1 <= t <= 3:
                emit_norm(14, t - 1)
            if t == 6:
                emit_outproj(13, act_only=True)
        for step in range(3):
            emit_norm(15, step)
        emit_outproj(13, act_only=True)
        emit_outproj(14)
        emit_outproj(15)


def get_program():
    global _PROGRAM
    if _PROGRAM is None:
        _PROGRAM = _build_program()
    return _PROGRAM


def prepare_in_maps(q, k, v, Wq, bq, Wk, bk, Wv, bv, Wo, bo):
    """Build the 8 per-core input dicts (host-side slicing/transposes)."""
    q = np.asarray(q, dtype=np.float32)
    k = np.asarray(k, dtype=np.float32)
    v = np.asarray(v, dtype=np.float32)
    xT = {}
    for b in range(B):
        xT[("q", b)] = np.ascontiguousarray(q[b].T).astype(np.float16)
        xT[("k", b)] = np.ascontiguousarray(k[b].T).astype(np.float16)
        xT[("v", b)] = np.ascontiguousarray(v[b].T).astype(np.float16)
    ones_v = np.ones((128, LT, 4, HD), dtype=np.float16)
    ident_m = np.eye(128, dtype=np.float16)
    in_maps = []
    for c in range(N_CORES):
        hg, b = c // 2, c % 2
        hs = hg * HS
        bq_s = np.asarray(bq, np.float32)[hs : hs + HS]
        bk_s = np.asarray(bk, np.float32)[hs : hs + HS]
        bv_s = np.asarray(bv, np.float32)[hs : hs + HS]
        bqkv_m = np.stack(
            [
                bq_s[0:128],
                bq_s[128:256],
                bk_s[0:128],
                bk_s[128:256],
                bv_s[0:128],
                bv_s[128:256],
            ],
            axis=1,
        )
        in_maps.append(
            {
                "xqT": xT[("q", b)],
                "xkT": xT[("k", b)],
                "xvT": xT[("v", b)],
                "wqT": np.asarray(Wq, np.float32)[hs : hs + HS, :].T.astype(np.float16),
                "wkT": np.asarray(Wk, np.float32)[hs : hs + HS, :].T.astype(np.float16),
                "wvT": np.asarray(Wv, np.float32)[hs : hs + HS, :].T.astype(np.float16),
                "woT": np.asarray(Wo, np.float32)[:, hs : hs + HS].T.astype(np.float16),
                "bqkv": np.ascontiguousarray(bqkv_m),
                "onesv": ones_v,
                "ident": ident_m,
            }
        )
    return in_maps


def combine_outputs(results, bo):
    """Sum head-group partials per batch and add the output bias."""
    bo = np.asarray(bo, np.float32)
    full = np.zeros((B, L, D), dtype=np.float32)
    for c in range(N_CORES):
        hg, b = c // 2, c % 2
        full[b] += np.asarray(results[c]["out"], dtype=np.float32)
    full += bo
    return full


def run(inputs, trace=False, trace_cores=None):
    nc = get_program()
    in_maps = prepare_in_maps(**inputs)
    res = run_bass_kernel_spmd(
        nc,
        in_maps,
        core_ids=list(range(N_CORES)),
        trace=trace,
        trace_cores=trace_cores,
    )
    out = combine_outputs(res.results, inputs["bo"])
    return out, res


def kernel(**inputs):
    out, _ = run(inputs, trace=False)
    return out
